# revision 59
# baseline (speedup 1.0000x reference)
"""Trainium2 Bass kernel: causal self-attention with RoPE.

Sharding: batch x head-quad. 2 batches x 4 core-groups = 8 cores; each core
handles one batch element and 4 heads (= 2 head-pair groups g=0,1). Each core
computes q/k/v projections for its 4 heads from its batch's tokens, runs
causal attention, and applies its 256-row slice of the output projection,
producing a partial [S, E] output in bf16. The host sums the 4 partials per
batch (the "all-reduce"). Versus head-only sharding this halves both the
input DMA (4MB) and the output partial (and bf16 partials halve it again).

Device-side layout choices:
  - x is passed pre-transposed ([E, S], bf16) so projections need no
    on-device transpose.
  - q and k are produced "d-major" (qT [128, g*S]); scores are computed
    transposed (S_T[k, q] = k_tile @ qT) so that P@V needs no transposes:
    O_T = [v | 1].T @ P_T, which also yields the softmax denominator as
    row 64 of the PSUM accumulator. Softmax uses no max-subtraction (max
    logit ~11 for this problem, exp is safe in fp32).
  - The scalar engine runs ONLY Exp: one activation table load for the
    whole kernel (table reloads cost ~1.3us each). All copies are DVE;
    normalization reciprocal is DVE (full-width, after broadcasting the
    raw denominator through a DRAM bounce), tri-mask is GpSimd.
  - Both heads' score blocks live in one 2-bank PSUM tile so the exp runs
    as a single fused op over [128, 2, nj].
  - RoPE: q' = q * cos + shuffle(q) * sin_signed (stream_shuffle swaps
    adjacent partitions; the sign lives in the host sin table).
  - Causal masking: after exp, the diagonal-crossing 128-wide region is
    multiplied by a 0/1 bf16 triangular mask on GpSimd; fully-masked
    columns are never computed.
  - V projection is fused across both head-pair groups (free dim 256,
    half the matmul instructions).
  - The out-projection contracts both groups in one PSUM chain and its
    chains are used as tensor-engine filler inside later attention
    chunks; attention interleaves the two groups largest-chunk-first so
    chains become ready early and the kernel ends on the smallest chunk.
    Dense PE occupancy keeps the HAM clock at 2.4GHz.
"""

import functools

import numpy as np
import ml_dtypes

import concourse.bass as bass
import concourse.mybir as mybir
import concourse.tile as tile
from concourse import bacc
from concourse.bass_utils import run_bass_kernel_spmd

F32 = mybir.dt.float32
BF16 = mybir.dt.bfloat16
BF = ml_dtypes.bfloat16

E = 1024
HD = 64
N_CORES = 8
NG = 2            # head-pair groups per core (4 heads = 2 pairs)
ROPE_BASE = 10000.0
EXPK = float(128.0 / np.log(2.0))   # folded into wq for Schraudolph exp


def _build(seq: int) -> bacc.Bacc:
    QC = min(512, seq)            # q-chunk width for attention
    NQC = seq // QC               # q-chunks per group
    NKTB = seq // 128             # k-tiles per group
    NET = E // 128                # contraction tiles = 8
    PCB = min(512, seq)           # projection s-chunk
    NPCB = seq // PCB
    NSTB = seq // 128             # token 128-blocks

    nc = bacc.Bacc(
        "TRN2",
        target_bir_lowering=False,
        debug=False,
        enable_asserts=False,
        num_devices=N_CORES,
    )

    xT_d = nc.dram_tensor("xT", [E, seq], BF16, kind="ExternalInput").ap()
    wq_d = nc.dram_tensor("wqT", [E, 256], BF16, kind="ExternalInput").ap()
    wk_d = nc.dram_tensor("wkT", [E, 256], BF16, kind="ExternalInput").ap()
    wv_d = nc.dram_tensor("wvT", [E, 256], BF16, kind="ExternalInput").ap()
    wo_d = nc.dram_tensor("woT", [256, E], BF16, kind="ExternalInput").ap()
    cos_d = nc.dram_tensor("cosT", [128, seq], BF16, kind="ExternalInput").ap()
    sin_d = nc.dram_tensor("sinT", [128, seq], BF16, kind="ExternalInput").ap()
    tri_d = nc.dram_tensor("tri", [128, 128], BF16, kind="ExternalInput").ap()
    out_d = nc.dram_tensor("out_p", [seq, E], BF16, kind="ExternalOutput").ap()
    recd = nc.dram_tensor("rec_scratch", [NG * NQC, 2 * QC], F32).ap()

    with tile.TileContext(nc) as tc:
        with (
            tc.tile_pool(name="persist", bufs=1) as persist,
            tc.tile_pool(name="pt", bufs=NKTB + 2) as ptp,
            tc.tile_pool(name="ob", bufs=4) as obp,
            tc.tile_pool(name="rec", bufs=4) as recp,
            tc.tile_pool(name="ps_big", bufs=2, space="PSUM") as psb,
            tc.tile_pool(name="ps_ov", bufs=2, space="PSUM") as psov,
            tc.tile_pool(name="ps_e", bufs=2, space="PSUM") as pse,
        ):
            def T(shape, dtype, name):
                return persist.tile(shape, dtype, name=name, tag=name)

            # ---- constants / weights
            wq_s = T([128, NET, 256], BF16, "wq_s")
            wk_s = T([128, NET, 256], BF16, "wk_s")
            wv_s = T([128, NET, 256], BF16, "wv_s")
            wo_s = T([128, NG, E], BF16, "wo_s")
            cos_s = T([128, seq], BF16, "cos_s")
            sin_s = T([128, seq], BF16, "sin_s")
            tri_s = T([128, 128], BF16, "tri_s")
            ones_s = T([1, 64], BF16, "ones_s")
            nc.gpsimd.memset(ones_s, 1.0)
            # warm-up source memset: first DVE instruction, so the PE
            # warm-up can begin right after engine init (~7.5us), before
            # any DMA data lands. fp32 so each warm-up matmul covers 4x
            # the cycles (fewer queue slots ahead of real work).
            wsrc = T([128, QC], F32, "wsrc")
            nc.vector.memset(wsrc, 0.0)
            # weights first on the two bulk queues; tables on gpsimd so
            # cos/sin are resident before the first rope
            nc.scalar.dma_start(out=wq_s, in_=wq_d.rearrange("(t p) d -> p t d", p=128))
            nc.sync.dma_start(out=wk_s, in_=wk_d.rearrange("(t p) d -> p t d", p=128))
            nc.gpsimd.dma_start(out=cos_s, in_=cos_d)
            nc.gpsimd.dma_start(out=sin_s, in_=sin_d)
            nc.gpsimd.dma_start(out=wv_s, in_=wv_d.rearrange("(t p) d -> p t d", p=128))
            nc.gpsimd.dma_start(out=tri_s, in_=tri_d)

            # ---- PE warm-up while input DMAs stream (HAM ramps at ~3.4us
            # of sustained activity; dummy matmuls buy 2.4GHz for the
            # projection phase). Gated only on the wq DMA (~3us).
            wu = psb.tile([128, 2, QC], F32, tag="psb", name="warmup")

            def warm(n):
                for _ in range(n):
                    nc.tensor.matmul(
                        wu[:, 0, :], lhsT=wsrc[:, 0:128], rhs=wsrc,
                        start=True, stop=True)

            # ~16us of fp32 matmuls: holds the HAM clock high until the
            # x bulk DMA fully lands (~24us) so the projection phase
            # runs at 2.4GHz from the start
            warm(18)

            # ---- resident input: one [128, seq] tile per E-block (4KB
            # DMA packets), split over the scalar and sync DMA queues so
            # both rings stream concurrently. The projection chains
            # consume E-blocks in arrival order, so compute starts as
            # soon as the first block lands.
            xrows = {}
            for et in range(NET):
                xt = T([128, seq], BF16, f"xr{et}")
                eng = nc.scalar if et % 2 == 0 else nc.sync
                eng.dma_start(
                    out=xt, in_=xT_d[et * 128:(et + 1) * 128, :])
                xrows[et] = xt
            nc.sync.dma_start(out=wo_s, in_=wo_d.rearrange("(g p) e -> p g e", p=128))
            xts = {}
            for pc in range(NPCB):
                for et in range(NET):
                    xts[(et, pc)] = xrows[et][:, pc * PCB:(pc + 1) * PCB]

            qT = T([128, NG * seq], BF16, "qT")
            kT = T([128, NG * seq], BF16, "kT")
            vo = T([128, NG * NKTB, 130], BF16, "vo")  # [vA|1|vB|1] per k-tile
            oT = T([128, NG * seq], BF16, "oT")
            nc.gpsimd.memset(vo, 1.0)

            # ---------- emission helpers ----------
            def rope(g, pc0, width):
                """RoPE over [pc0*PCB, pc0*PCB + width) token columns."""
                for t, nm in ((qT, "q"), (kT, "k")):
                    cols = slice(g * seq + pc0 * PCB,
                                 g * seq + pc0 * PCB + width)
                    tcols = slice(pc0 * PCB, pc0 * PCB + width)
                    sh = recp.tile([128, 2 * PCB], BF16, tag="ropesh",
                                   name=f"sh{nm}{g}_{pc0}")
                    shw = sh[:, 0:width]
                    nc.vector.stream_shuffle(
                        shw, t[:, cols], [i ^ 1 for i in range(32)])
                    nc.vector.tensor_mul(shw, shw, sin_s[:, tcols])
                    nc.vector.tensor_mul(t[:, cols], t[:, cols], cos_s[:, tcols])
                    nc.vector.tensor_add(t[:, cols], t[:, cols], shw)

            def proj_qk_pieces(g, pc):
                """Micro-tasks (~2 MMs each) for one q/k projection chunk."""
                cols = slice(g * seq + pc * PCB, g * seq + (pc + 1) * PCB)
                pieces = []
                state = {}
                for wi, (w_s, dst) in enumerate(((wq_s, qT), (wk_s, kT))):
                    for e0 in range(0, NET, 2):
                        def piece(wi=wi, w_s=w_s, dst=dst, e0=e0):
                            if e0 == 0:
                                state[wi] = pse.tile(
                                    [128, PCB], F32, tag="pse",
                                    name=f"qk{g}_{pc}_{wi}")
                            ps = state[wi]
                            for et in (e0, e0 + 1):
                                nc.tensor.matmul(
                                    ps, lhsT=w_s[:, et, g * 128:g * 128 + 128],
                                    rhs=xts[(et, pc)],
                                    start=(et == 0), stop=(et == NET - 1),
                                )
                            if e0 + 2 == NET:
                                # scalar: it has slack in both windows
                                # these run in (DVE carries rope + exp)
                                nc.scalar.copy(out=dst[:, cols], in_=ps)
                                if wi == 1 and pc % 2 == 1:
                                    rope(g, pc - 1, 2 * PCB)
                        pieces.append(piece)
                return pieces

            def proj_qk(g, pc):
                for p in proj_qk_pieces(g, pc):
                    p()

            def proj_v_pieces(g, st):
                """V projection for token block st, one group (2 heads)."""
                state = {}
                pieces = []
                for e0 in range(0, NET, 4):
                    def piece(e0=e0):
                        if e0 == 0:
                            state[0] = pse.tile([128, PCB], F32, tag="pse",
                                                name=f"v{g}_{st}")
                        ps = state[0]
                        per = PCB // 128
                        vpc = st // per
                        vc0 = (st % per) * 128
                        for et in range(e0, e0 + 4):
                            nc.tensor.matmul(
                                ps[:, 0:128],
                                lhsT=xts[(et, vpc)][:, vc0:vc0 + 128],
                                rhs=wv_s[:, et, g * 128:g * 128 + 128],
                                start=(et == 0), stop=(et == NET - 1),
                            )
                        if e0 + 4 == NET:
                            base = vo[:, g * NKTB + st, :]
                            dst = bass.AP(
                                tensor=base.tensor, offset=base.offset,
                                ap=[list(base.ap[0]), [65, 2], [1, 64]])
                            src = ps[:, 0:128].rearrange(
                                "p (h d) -> p h d", d=64)
                            nc.scalar.copy(out=dst, in_=src)
                    pieces.append(piece)
                return pieces

            def proj_v(g, st):
                for p in proj_v_pieces(g, st):
                    p()

            pts_cache = {}

            def d1_kj(g, c, kj):
                qbase = c * QC
                gq0 = g * seq + qbase
                o = kj * 128 - qbase
                ro = max(o, 0)
                nj = QC - ro
                kc = g * seq + kj * 128
                ps = psb.tile([128, 2, QC], F32, tag="psb",
                              name=f"ss{g}_{c}_{kj}")
                for h in range(2):
                    rows = slice(h * 64, h * 64 + 64)
                    nc.tensor.matmul(
                        ps[:, h, 0:nj],
                        lhsT=kT[rows, kc:kc + 128],
                        rhs=qT[rows, gq0 + ro:gq0 + QC],
                        start=True, stop=True,
                        tile_position=(h * 64, 0),
                    )
                pt = ptp.tile([128, 2, QC], BF16, tag="pt",
                              name=f"pt{g}_{c}_{kj}")
                # exp is the attention bottleneck: split it across the
                # scalar engine (true exp; EXPK is pre-folded into wq so
                # scale un-folds it) and the DVE (Schraudolph: the psum
                # already holds x*128/ln2, so  bf16(exp(x)) ~=
                # bitcast_int16(x*128/ln2 + B)  in one tensor_scalar op).
                if c >= 2 and kj % 3 == 2:
                    nc.vector.tensor_scalar(
                        out=pt[:, :, 0:nj].bitcast(mybir.dt.int16),
                        in0=ps[:, :, 0:nj],
                        scalar1=16250.75, scalar2=None,
                        op0=mybir.AluOpType.add,
                    )
                else:
                    nc.scalar.activation(
                        pt[:, :, 0:nj], ps[:, :, 0:nj],
                        mybir.ActivationFunctionType.Exp, scale=1.0 / EXPK,
                    )
                if o >= 0:
                    tri_b = bass.AP(
                        tensor=tri_s.tensor, offset=tri_s.offset,
                        ap=[list(tri_s.ap[0]), [0, 2], list(tri_s.ap[1])],
                    )
                    nc.gpsimd.tensor_mul(
                        pt[:, :, 0:128], pt[:, :, 0:128], tri_b)
                return pt, ro, nj

            def d2_kj(g, c, kj, ops_, nkt):
                pt, ro, nj = pts_cache[(g, c, kj)]
                for h in range(2):
                    nc.tensor.matmul(
                        ops_[h][:, ro:QC],
                        lhsT=vo[:, g * NKTB + kj, h * 65:h * 65 + 65],
                        rhs=pt[:, h, 0:nj],
                        start=(kj == 0), stop=(kj == nkt - 1),
                    )

            def attn_body(g, c, fq, lateq=(), start_kj=0):
                """d1 loop of one chunk starting at start_kj (earlier kjs
                pre-emitted by the caller for cross-chunk overlap). Does
                NOT emit the final two d2s — the caller interleaves them
                with the next chunk's first d1s.
                fq: fillers spread evenly over the loop (projection work
                needed by upcoming chunks). lateq: fillers packed into
                the second half (out-proj chains — they cover the
                exp-bound tail)."""
                qbase = c * QC
                nkt = (qbase + QC) // 128
                lateq = list(lateq)
                ops_ = [psov.tile([65, QC], F32, tag="psov",
                                  name=f"o{g}_{c}_{h}")
                        for h in range(2)]
                for kj in range(start_kj, nkt):
                    pt, ro, nj = d1_kj(g, c, kj)
                    if kj >= 2:
                        d2_kj(g, c, kj - 2, ops_, nkt)
                    npop = -(-len(fq) // (nkt - kj))
                    for _ in range(npop):
                        if fq:
                            fq.pop(0)()
                    if kj >= nkt // 2:
                        npop = -(-len(lateq) // (nkt - kj))
                        for _ in range(npop):
                            if lateq:
                                lateq.pop(0)()
                    pts_cache[(g, c, kj)] = (pt, ro, nj)
                # leftover fillers run while the last exps drain
                for p in fq:
                    p()
                del fq[:]
                for p in lateq:
                    p()
                return ops_, nkt

            def d3_norm(g, c, ops_, last=False):
                """oT <- O rows; normalization: raw denominator row ->
                DRAM bounce broadcast (scalar DMA queue, uncontended) ->
                DVE reciprocal -> GpSimd multiply. For the last chunk the
                broadcast goes through a K=1 matmul instead (the PE is
                idle there and the DMA roundtrip would serialize the
                tail)."""
                gq0 = g * seq + c * QC
                ridx = g * NQC + c
                dch = recp.tile([1, 2, QC], F32, tag="dch", name=f"dc{g}_{c}")
                for h in range(2):
                    nc.vector.tensor_copy(
                        out=dch[:, h, :], in_=ops_[h][64:65, 0:QC])
                for h in range(2):
                    nc.vector.tensor_copy(
                        out=oT[h * 64:h * 64 + 64, gq0:gq0 + QC],
                        in_=ops_[h][0:64, 0:QC])
                if last:
                    rc = recp.tile([1, 2, QC], F32, tag="rcf", name=f"rf{g}_{c}")
                    nc.vector.reciprocal_approx_fast(rc, dch)
                    rcb = recp.tile([1, 2, QC], BF16, tag="rcb",
                                    name=f"rb{g}_{c}")
                    nc.vector.tensor_copy(out=rcb, in_=rc)
                    for h in range(2):
                        rb = psov.tile([65, QC], F32, tag="psov",
                                       name=f"rp{g}_{c}_{h}")
                        nc.tensor.matmul(
                            rb[0:64, :], lhsT=ones_s, rhs=rcb[:, h, :],
                            start=True, stop=True)
                        nc.vector.tensor_mul(
                            oT[h * 64:h * 64 + 64, gq0:gq0 + QC],
                            oT[h * 64:h * 64 + 64, gq0:gq0 + QC],
                            rb[0:64, :])
                    return
                nc.sync.dma_start(
                    out=recd[ridx:ridx + 1, :],
                    in_=dch.rearrange("p h q -> p (h q)"))
                den = recp.tile([128, QC], F32, tag="den", name=f"db{g}_{c}")
                for h in range(2):
                    row = recd[ridx:ridx + 1, h * QC:(h + 1) * QC]
                    bcast = bass.AP(tensor=row.tensor, offset=row.offset,
                                    ap=[[0, 64], [1, QC]])
                    nc.sync.dma_start(out=den[h * 64:h * 64 + 64, :],
                                      in_=bcast)
                rr = recp.tile([128, QC], F32, tag="rr", name=f"rr{g}_{c}")
                nc.vector.reciprocal_approx_fast(rr, den)
                nc.gpsimd.tensor_mul(
                    oT[:, gq0:gq0 + QC], oT[:, gq0:gq0 + QC], rr)

            def eproj_pieces(tb, at_tail=False):
                """Out-projection chains for token block tb (2 MMs each,
                contracting both groups; bf16 partial to DRAM). At the
                tail the psum->sbuf casts alternate DVE/scalar (the exp
                stream is finished, the scalar engine is idle) and the
                writes spread over all three DMA queues."""
                def chain(tb, ec):
                    ps = pse.tile([128, PCB], F32, tag="pse",
                                  name=f"op{tb}_{ec}")
                    for g in range(NG):
                        nc.tensor.matmul(
                            ps[:, 0:512],
                            lhsT=oT[:, g * seq + tb * 128:
                                    g * seq + (tb + 1) * 128],
                            rhs=wo_s[:, g, ec * 512:(ec + 1) * 512],
                            start=(g == 0), stop=(g == NG - 1),
                        )
                    ob = obp.tile([128, 512], BF16, tag="ob",
                                  name=f"ob{tb}_{ec}")
                    if (tb * 2 + ec) % 2:
                        nc.scalar.copy(out=ob, in_=ps[:, 0:512])
                    else:
                        nc.vector.tensor_copy(out=ob, in_=ps[:, 0:512])
                    if at_tail:
                        eng = (nc.sync, nc.scalar, nc.gpsimd)[(tb * 2 + ec) % 3]
                    else:
                        eng = (nc.sync, nc.gpsimd)[(tb * 2 + ec) % 2]
                    eng.dma_start(
                        out=out_d[tb * 128:(tb + 1) * 128,
                                  ec * 512:(ec + 1) * 512],
                        in_=ob,
                    )
                return [lambda ec=ec: chain(tb, ec) for ec in range(E // 512)]

            # ---------- emission ----------
            for pc in range(NPCB):
                proj_qk(0, pc)
            for st in range(NSTB // 2):
                proj_v(0, st)

            # attention schedule: small chunks first (their PE deficit is
            # covered by g1 projection fillers while the exp stream is
            # short), big exp-dense chunks last (their deep d1 loops give
            # the den-roundtrip of earlier norms plenty of slack before
            # the out-proj chains consume oT). Chains for chunk c are
            # placed >= 2 chunks after (1, c) so they never stall the
            # in-order PE queue.
            spc = QC // 128
            sched = [(0, 1), (1, 1), (0, 0), (1, 0),
                     (0, 3), (1, 3), (0, 2), (1, 2)]
            if NQC != 4:
                sched = []
                for c in reversed(range(NQC)):
                    sched.append((0, c))
                    sched.append((1, c))
            fq_map = {s: [] for s in sched}
            for pc in range(NPCB):
                fq_map[sched[0]].extend(proj_qk_pieces(1, pc))
            for st in range(NSTB):
                fq_map[sched[1]].extend(proj_v_pieces(1, st))
            # v-g0 upper half: only needed from chunk c=2 on; fill the
            # small chunks' thin queues with it
            if NQC == 4:
                for st in range(NSTB // 2, 3 * NSTB // 4):
                    fq_map[(0, 0)].extend(proj_v_pieces(0, st))
                for st in range(3 * NSTB // 4, NSTB):
                    fq_map[(1, 0)].extend(proj_v_pieces(0, st))
            else:
                for st in range(NSTB // 2, NSTB):
                    fq_map[sched[0]].extend(proj_v_pieces(0, st))

            def chains_of(c, at_tail=False):
                out = []
                for tb in range(c * spc, (c + 1) * spc):
                    out.extend(eproj_pieces(tb, at_tail))
                return out

            late_map = {s: [] for s in sched}
            tail = []
            if NQC == 4:
                late_map[(0, 3)].extend(chains_of(1))
                late_map[(1, 3)].extend(chains_of(0))
                late_map[(1, 2)].extend(chains_of(3))
                tail = chains_of(2, at_tail=True)
            else:
                for i, s in enumerate(sched):
                    g, c = s
                    if g != 1:
                        continue
                    chains = chains_of(c)
                    targets = sched[i + 2:i + 4]
                    if not targets:
                        tail.extend(chains)
                    else:
                        per = (len(chains) + len(targets) - 1) // len(targets)
                        for j, ns in enumerate(targets):
                            late_map[ns].extend(chains[j * per:(j + 1) * per])

            # chunk-boundary overlap: the next chunk's first two d1s are
            # emitted BEFORE the previous chunk's final two d2s (which
            # wait on the last exps), so the in-order PE queue never
            # idles across boundaries.
            prev = None
            for s in sched + [None]:
                if s is not None:
                    g, c = s
                    nkt_s = (c * QC + QC) // 128
                    fqs = fq_map[s]
                    for kj in range(2):
                        pts_cache[(g, c, kj)] = d1_kj(g, c, kj)
                        # pop this iteration's filler share too (fillers
                        # can feed this chunk's own d2 stream, e.g. the
                        # g1 V projection)
                        npop = -(-len(fqs) // (nkt_s - kj))
                        for _ in range(npop):
                            if fqs:
                                fqs.pop(0)()
                if prev is not None:
                    pg, pc_, pops, pnkt = prev
                    for kj in range(max(pnkt - 2, 0), pnkt):
                        d2_kj(pg, pc_, kj, pops, pnkt)
                    d3_norm(pg, pc_, pops, last=(s is None))
                if s is None:
                    break
                ops_, nkt = attn_body(g, c, fq_map[s], late_map[s],
                                      start_kj=2)
                prev = (g, c, ops_, nkt)
            for p in tail:
                p()

    nc.compile()
    return nc


@functools.lru_cache(maxsize=2)
def _built(seq: int) -> bacc.Bacc:
    return _build(seq)


def _host_tables(seq: int):
    inv = 1.0 / (ROPE_BASE ** (np.arange(0, HD, 2, dtype=np.float32) / HD))
    f = np.outer(np.arange(seq, dtype=np.float32), inv)
    emb = np.concatenate([f, f], axis=-1)        # [S, 64] (concat layout)
    cos = np.cos(emb).T.astype(np.float32)       # [64, S]
    sin = np.sin(emb).T.astype(np.float32)
    sgn = np.where(np.arange(HD) % 2 == 0, -1.0, 1.0).astype(np.float32)
    sin_signed = sin * sgn[:, None]
    cosT = np.concatenate([cos, cos], axis=0).astype(BF)       # [128, S]
    sinT = np.concatenate([sin_signed, sin_signed], axis=0).astype(BF)
    return cosT, sinT


def make_in_maps(x, Wq, Wk, Wv, Wo):
    x = np.asarray(x, dtype=np.float32)
    B, S, E_ = x.shape
    assert E_ == E
    xTs = [np.ascontiguousarray(x[b].T).astype(BF) for b in range(B)]  # [E,S]
    cosT, sinT = _host_tables(S)
    i_idx = np.arange(128)
    tri = (i_idx[None, :] >= i_idx[:, None]).astype(BF)  # keep j >= i
    scale = np.float32(HD ** -0.5 * EXPK)
    in_maps = []
    for core in range(N_CORES):
        b = core // 4
        m = core % 4
        rows = slice(m * 256, m * 256 + 256)     # this core's 4 heads' dims
        wqT = np.ascontiguousarray((np.asarray(Wq)[rows, :] * scale).T).astype(BF)
        wkT = np.ascontiguousarray(np.asarray(Wk)[rows, :].T).astype(BF)
        wvT = np.ascontiguousarray(np.asarray(Wv)[rows, :].T).astype(BF)
        woT = np.ascontiguousarray(np.asarray(Wo)[:, rows].T).astype(BF)
        in_maps.append(dict(
            xT=xTs[b], wqT=wqT, wkT=wkT, wvT=wvT, woT=woT,
            cosT=cosT, sinT=sinT, tri=tri,
        ))
    return in_maps


def kernel(x, Wq, Wk, Wv, Wo):
    x = np.asarray(x, dtype=np.float32)
    B, S, E_ = x.shape
    nc = _built(S)
    in_maps = make_in_maps(x, Wq, Wk, Wv, Wo)
    res = run_bass_kernel_spmd(nc, in_maps, core_ids=list(range(N_CORES)))
    out = np.zeros((B, S, E_), np.float32)
    for core, r in enumerate(res.results):
        out[core // 4] += r["out_p"].astype(np.float32)
    return out


# revision 60
# speedup vs baseline: 1.0130x; 1.0130x over previous
"""Trainium2 Bass kernel: causal self-attention with RoPE.

Sharding: batch x head-quad. 2 batches x 4 core-groups = 8 cores; each core
handles one batch element and 4 heads (= 2 head-pair groups g=0,1). Each core
computes q/k/v projections for its 4 heads from its batch's tokens, runs
causal attention, and applies its 256-row slice of the output projection,
producing a partial [S, E] output in bf16. The host sums the 4 partials per
batch (the "all-reduce"). Versus head-only sharding this halves both the
input DMA (4MB) and the output partial (and bf16 partials halve it again).

Device-side layout choices:
  - x is passed pre-transposed ([E, S], bf16) so projections need no
    on-device transpose.
  - q and k are produced "d-major" (qT [128, g*S]); scores are computed
    transposed (S_T[k, q] = k_tile @ qT) so that P@V needs no transposes:
    O_T = [v | 1].T @ P_T, which also yields the softmax denominator as
    row 64 of the PSUM accumulator. Softmax uses no max-subtraction (max
    logit ~11 for this problem, exp is safe in fp32).
  - The scalar engine runs ONLY Exp: one activation table load for the
    whole kernel (table reloads cost ~1.3us each). All copies are DVE;
    normalization reciprocal is DVE (full-width, after broadcasting the
    raw denominator through a DRAM bounce), tri-mask is GpSimd.
  - Both heads' score blocks live in one 2-bank PSUM tile so the exp runs
    as a single fused op over [128, 2, nj].
  - RoPE: q' = q * cos + shuffle(q) * sin_signed (stream_shuffle swaps
    adjacent partitions; the sign lives in the host sin table).
  - Causal masking: after exp, the diagonal-crossing 128-wide region is
    multiplied by a 0/1 bf16 triangular mask on GpSimd; fully-masked
    columns are never computed.
  - V projection is fused across both head-pair groups (free dim 256,
    half the matmul instructions).
  - The out-projection contracts both groups in one PSUM chain and its
    chains are used as tensor-engine filler inside later attention
    chunks; attention interleaves the two groups largest-chunk-first so
    chains become ready early and the kernel ends on the smallest chunk.
    Dense PE occupancy keeps the HAM clock at 2.4GHz.
"""

import functools

import numpy as np
import ml_dtypes

import concourse.bass as bass
import concourse.mybir as mybir
import concourse.tile as tile
from concourse import bacc
from concourse.bass_utils import run_bass_kernel_spmd

F32 = mybir.dt.float32
BF16 = mybir.dt.bfloat16
BF = ml_dtypes.bfloat16

E = 1024
HD = 64
N_CORES = 8
NG = 2            # head-pair groups per core (4 heads = 2 pairs)
ROPE_BASE = 10000.0
EXPK = float(128.0 / np.log(2.0))   # folded into wq for Schraudolph exp


def _build(seq: int) -> bacc.Bacc:
    QC = min(512, seq)            # q-chunk width for attention
    NQC = seq // QC               # q-chunks per group
    NKTB = seq // 128             # k-tiles per group
    NET = E // 128                # contraction tiles = 8
    PCB = min(512, seq)           # projection s-chunk
    NPCB = seq // PCB
    NSTB = seq // 128             # token 128-blocks

    nc = bacc.Bacc(
        "TRN2",
        target_bir_lowering=False,
        debug=False,
        enable_asserts=False,
        num_devices=N_CORES,
    )

    xT_d = nc.dram_tensor("xT", [E, seq], BF16, kind="ExternalInput").ap()
    wq_d = nc.dram_tensor("wqT", [E, 256], BF16, kind="ExternalInput").ap()
    wk_d = nc.dram_tensor("wkT", [E, 256], BF16, kind="ExternalInput").ap()
    wv_d = nc.dram_tensor("wvT", [E, 256], BF16, kind="ExternalInput").ap()
    wo_d = nc.dram_tensor("woT", [256, E], BF16, kind="ExternalInput").ap()
    cos_d = nc.dram_tensor("cosT", [128, seq], BF16, kind="ExternalInput").ap()
    sin_d = nc.dram_tensor("sinT", [128, seq], BF16, kind="ExternalInput").ap()
    tri_d = nc.dram_tensor("tri", [128, 128], BF16, kind="ExternalInput").ap()
    out_d = nc.dram_tensor("out_p", [seq, E], BF16, kind="ExternalOutput").ap()
    recd = nc.dram_tensor("rec_scratch", [NG * NQC, 2 * QC], F32).ap()

    with tile.TileContext(nc) as tc:
        with (
            tc.tile_pool(name="persist", bufs=1) as persist,
            tc.tile_pool(name="pt", bufs=NKTB + 2) as ptp,
            tc.tile_pool(name="ob", bufs=4) as obp,
            tc.tile_pool(name="rec", bufs=4) as recp,
            tc.tile_pool(name="ps_big", bufs=2, space="PSUM") as psb,
            tc.tile_pool(name="ps_ov", bufs=2, space="PSUM") as psov,
            tc.tile_pool(name="ps_e", bufs=2, space="PSUM") as pse,
        ):
            def T(shape, dtype, name):
                return persist.tile(shape, dtype, name=name, tag=name)

            # ---- constants / weights
            wq_s = T([128, NET, 256], BF16, "wq_s")
            wk_s = T([128, NET, 256], BF16, "wk_s")
            wv_s = T([128, NET, 256], BF16, "wv_s")
            wo_s = T([128, NG, E], BF16, "wo_s")
            cos_s = T([128, seq], BF16, "cos_s")
            sin_s = T([128, seq], BF16, "sin_s")
            tri_s = T([128, 128], BF16, "tri_s")
            ones_s = T([1, 64], BF16, "ones_s")
            nc.gpsimd.memset(ones_s, 1.0)
            # warm-up source memset: first DVE instruction, so the PE
            # warm-up can begin right after engine init (~7.5us), before
            # any DMA data lands. fp32 so each warm-up matmul covers 4x
            # the cycles (fewer queue slots ahead of real work).
            wsrc = T([128, QC], F32, "wsrc")
            nc.vector.memset(wsrc, 0.0)
            # weights first on the two bulk queues; tables on gpsimd so
            # cos/sin are resident before the first rope
            nc.scalar.dma_start(out=wq_s, in_=wq_d.rearrange("(t p) d -> p t d", p=128))
            nc.sync.dma_start(out=wk_s, in_=wk_d.rearrange("(t p) d -> p t d", p=128))
            nc.gpsimd.dma_start(out=cos_s, in_=cos_d)
            nc.gpsimd.dma_start(out=sin_s, in_=sin_d)
            nc.gpsimd.dma_start(out=wv_s, in_=wv_d.rearrange("(t p) d -> p t d", p=128))
            nc.gpsimd.dma_start(out=tri_s, in_=tri_d)

            # ---- PE warm-up while input DMAs stream (HAM ramps at ~3.4us
            # of sustained activity; dummy matmuls buy 2.4GHz for the
            # projection phase). Gated only on the wq DMA (~3us).
            wu = psb.tile([128, 2, QC], F32, tag="psb", name="warmup")

            def warm(n):
                for _ in range(n):
                    nc.tensor.matmul(
                        wu[:, 0, :], lhsT=wsrc[:, 0:128], rhs=wsrc,
                        start=True, stop=True)

            # ~16us of fp32 matmuls: holds the HAM clock high until the
            # x bulk DMA fully lands (~24us) so the projection phase
            # runs at 2.4GHz from the start
            warm(18)

            # ---- resident input: one [128, seq] tile per E-block (4KB
            # DMA packets), split over the scalar and sync DMA queues so
            # both rings stream concurrently. The projection chains
            # consume E-blocks in arrival order, so compute starts as
            # soon as the first block lands.
            xrows = {}
            for et in range(NET):
                xt = T([128, seq], BF16, f"xr{et}")
                eng = nc.scalar if et % 2 == 0 else nc.sync
                eng.dma_start(
                    out=xt, in_=xT_d[et * 128:(et + 1) * 128, :])
                xrows[et] = xt
            nc.sync.dma_start(out=wo_s, in_=wo_d.rearrange("(g p) e -> p g e", p=128))
            xts = {}
            for pc in range(NPCB):
                for et in range(NET):
                    xts[(et, pc)] = xrows[et][:, pc * PCB:(pc + 1) * PCB]

            qT = T([128, NG * seq], BF16, "qT")
            kT = T([128, NG * seq], BF16, "kT")
            vo = T([128, NG * NKTB, 130], BF16, "vo")  # [vA|1|vB|1] per k-tile
            oT = T([128, NG * seq], BF16, "oT")
            nc.gpsimd.memset(vo, 1.0)

            # ---------- emission helpers ----------
            def rope(g, pc0, width):
                """RoPE over [pc0*PCB, pc0*PCB + width) token columns."""
                for t, nm in ((qT, "q"), (kT, "k")):
                    cols = slice(g * seq + pc0 * PCB,
                                 g * seq + pc0 * PCB + width)
                    tcols = slice(pc0 * PCB, pc0 * PCB + width)
                    sh = recp.tile([128, 2 * PCB], BF16, tag="ropesh",
                                   name=f"sh{nm}{g}_{pc0}")
                    shw = sh[:, 0:width]
                    nc.vector.stream_shuffle(
                        shw, t[:, cols], [i ^ 1 for i in range(32)])
                    nc.vector.tensor_mul(shw, shw, sin_s[:, tcols])
                    nc.vector.tensor_mul(t[:, cols], t[:, cols], cos_s[:, tcols])
                    nc.vector.tensor_add(t[:, cols], t[:, cols], shw)

            def proj_qk_pieces(g, pc):
                """Micro-tasks (~2 MMs each) for one q/k projection chunk."""
                cols = slice(g * seq + pc * PCB, g * seq + (pc + 1) * PCB)
                pieces = []
                state = {}
                for wi, (w_s, dst) in enumerate(((wq_s, qT), (wk_s, kT))):
                    for e0 in range(0, NET, 2):
                        def piece(wi=wi, w_s=w_s, dst=dst, e0=e0):
                            if e0 == 0:
                                state[wi] = pse.tile(
                                    [128, PCB], F32, tag="pse",
                                    name=f"qk{g}_{pc}_{wi}")
                            ps = state[wi]
                            for et in (e0, e0 + 1):
                                nc.tensor.matmul(
                                    ps, lhsT=w_s[:, et, g * 128:g * 128 + 128],
                                    rhs=xts[(et, pc)],
                                    start=(et == 0), stop=(et == NET - 1),
                                )
                            if e0 + 2 == NET:
                                # scalar: it has slack in both windows
                                # these run in (DVE carries rope + exp)
                                nc.scalar.copy(out=dst[:, cols], in_=ps)
                                if wi == 1 and pc % 2 == 1:
                                    rope(g, pc - 1, 2 * PCB)
                        pieces.append(piece)
                return pieces

            def proj_qk(g, pc):
                for p in proj_qk_pieces(g, pc):
                    p()

            def proj_v_pieces(g, st):
                """V projection for token block st, one group (2 heads)."""
                state = {}
                pieces = []
                for e0 in range(0, NET, 4):
                    def piece(e0=e0):
                        if e0 == 0:
                            state[0] = pse.tile([128, PCB], F32, tag="pse",
                                                name=f"v{g}_{st}")
                        ps = state[0]
                        per = PCB // 128
                        vpc = st // per
                        vc0 = (st % per) * 128
                        for et in range(e0, e0 + 4):
                            nc.tensor.matmul(
                                ps[:, 0:128],
                                lhsT=xts[(et, vpc)][:, vc0:vc0 + 128],
                                rhs=wv_s[:, et, g * 128:g * 128 + 128],
                                start=(et == 0), stop=(et == NET - 1),
                            )
                        if e0 + 4 == NET:
                            base = vo[:, g * NKTB + st, :]
                            dst = bass.AP(
                                tensor=base.tensor, offset=base.offset,
                                ap=[list(base.ap[0]), [65, 2], [1, 64]])
                            src = ps[:, 0:128].rearrange(
                                "p (h d) -> p h d", d=64)
                            nc.scalar.copy(out=dst, in_=src)
                    pieces.append(piece)
                return pieces

            def proj_v(g, st):
                for p in proj_v_pieces(g, st):
                    p()

            pts_cache = {}

            def d1_kj(g, c, kj):
                qbase = c * QC
                gq0 = g * seq + qbase
                o = kj * 128 - qbase
                ro = max(o, 0)
                nj = QC - ro
                kc = g * seq + kj * 128
                ps = psb.tile([128, 2, QC], F32, tag="psb",
                              name=f"ss{g}_{c}_{kj}")
                for h in range(2):
                    rows = slice(h * 64, h * 64 + 64)
                    nc.tensor.matmul(
                        ps[:, h, 0:nj],
                        lhsT=kT[rows, kc:kc + 128],
                        rhs=qT[rows, gq0 + ro:gq0 + QC],
                        start=True, stop=True,
                        tile_position=(h * 64, 0),
                    )
                pt = ptp.tile([128, 2, QC], BF16, tag="pt",
                              name=f"pt{g}_{c}_{kj}")
                # exp is the attention bottleneck: split it across the
                # scalar engine (true exp; EXPK is pre-folded into wq so
                # scale un-folds it) and the DVE (Schraudolph: the psum
                # already holds x*128/ln2, so  bf16(exp(x)) ~=
                # bitcast_int16(x*128/ln2 + B)  in one tensor_scalar op).
                if c >= 2 and kj % 3 == 2:
                    nc.vector.tensor_scalar(
                        out=pt[:, :, 0:nj].bitcast(mybir.dt.int16),
                        in0=ps[:, :, 0:nj],
                        scalar1=16250.75, scalar2=None,
                        op0=mybir.AluOpType.add,
                    )
                else:
                    nc.scalar.activation(
                        pt[:, :, 0:nj], ps[:, :, 0:nj],
                        mybir.ActivationFunctionType.Exp, scale=1.0 / EXPK,
                    )
                if o >= 0:
                    tri_b = bass.AP(
                        tensor=tri_s.tensor, offset=tri_s.offset,
                        ap=[list(tri_s.ap[0]), [0, 2], list(tri_s.ap[1])],
                    )
                    nc.gpsimd.tensor_mul(
                        pt[:, :, 0:128], pt[:, :, 0:128], tri_b)
                return pt, ro, nj

            def d2_kj(g, c, kj, ops_, nkt):
                pt, ro, nj = pts_cache[(g, c, kj)]
                for h in range(2):
                    nc.tensor.matmul(
                        ops_[h][:, ro:QC],
                        lhsT=vo[:, g * NKTB + kj, h * 65:h * 65 + 65],
                        rhs=pt[:, h, 0:nj],
                        start=(kj == 0), stop=(kj == nkt - 1),
                    )

            def attn_body(g, c, fq, lateq=(), start_kj=0):
                """d1 loop of one chunk starting at start_kj (earlier kjs
                pre-emitted by the caller for cross-chunk overlap). Does
                NOT emit the final two d2s — the caller interleaves them
                with the next chunk's first d1s.
                fq: fillers spread evenly over the loop (projection work
                needed by upcoming chunks). lateq: fillers packed into
                the second half (out-proj chains — they cover the
                exp-bound tail)."""
                qbase = c * QC
                nkt = (qbase + QC) // 128
                lateq = list(lateq)
                ops_ = [psov.tile([65, QC], F32, tag="psov",
                                  name=f"o{g}_{c}_{h}")
                        for h in range(2)]
                for kj in range(start_kj, nkt):
                    pt, ro, nj = d1_kj(g, c, kj)
                    if kj >= 2:
                        d2_kj(g, c, kj - 2, ops_, nkt)
                    npop = -(-len(fq) // (nkt - kj))
                    for _ in range(npop):
                        if fq:
                            fq.pop(0)()
                    if kj >= nkt // 2:
                        npop = -(-len(lateq) // (nkt - kj))
                        for _ in range(npop):
                            if lateq:
                                lateq.pop(0)()
                    pts_cache[(g, c, kj)] = (pt, ro, nj)
                # leftover fillers run while the last exps drain
                for p in fq:
                    p()
                del fq[:]
                for p in lateq:
                    p()
                return ops_, nkt

            def d3_norm(g, c, ops_, last=False):
                """oT <- O rows; normalization: raw denominator row ->
                DRAM bounce broadcast (scalar DMA queue, uncontended) ->
                DVE reciprocal -> GpSimd multiply. For the last chunk the
                broadcast goes through a K=1 matmul instead (the PE is
                idle there and the DMA roundtrip would serialize the
                tail)."""
                gq0 = g * seq + c * QC
                ridx = g * NQC + c
                dch = recp.tile([1, 2, QC], F32, tag="dch", name=f"dc{g}_{c}")
                for h in range(2):
                    # scalar engine: keeps these partition-1 copies off
                    # the DVE, which carries part of the exp stream
                    nc.scalar.copy(
                        out=dch[:, h, :], in_=ops_[h][64:65, 0:QC])
                for h in range(2):
                    nc.vector.tensor_copy(
                        out=oT[h * 64:h * 64 + 64, gq0:gq0 + QC],
                        in_=ops_[h][0:64, 0:QC])
                if last:
                    rc = recp.tile([1, 2, QC], F32, tag="rcf", name=f"rf{g}_{c}")
                    nc.vector.reciprocal_approx_fast(rc, dch)
                    rcb = recp.tile([1, 2, QC], BF16, tag="rcb",
                                    name=f"rb{g}_{c}")
                    nc.vector.tensor_copy(out=rcb, in_=rc)
                    for h in range(2):
                        rb = psov.tile([65, QC], F32, tag="psov",
                                       name=f"rp{g}_{c}_{h}")
                        nc.tensor.matmul(
                            rb[0:64, :], lhsT=ones_s, rhs=rcb[:, h, :],
                            start=True, stop=True)
                        nc.vector.tensor_mul(
                            oT[h * 64:h * 64 + 64, gq0:gq0 + QC],
                            oT[h * 64:h * 64 + 64, gq0:gq0 + QC],
                            rb[0:64, :])
                    return
                nc.sync.dma_start(
                    out=recd[ridx:ridx + 1, :],
                    in_=dch.rearrange("p h q -> p (h q)"))
                den = recp.tile([128, QC], F32, tag="den", name=f"db{g}_{c}")
                for h in range(2):
                    row = recd[ridx:ridx + 1, h * QC:(h + 1) * QC]
                    bcast = bass.AP(tensor=row.tensor, offset=row.offset,
                                    ap=[[0, 64], [1, QC]])
                    nc.sync.dma_start(out=den[h * 64:h * 64 + 64, :],
                                      in_=bcast)
                rr = recp.tile([128, QC], F32, tag="rr", name=f"rr{g}_{c}")
                nc.vector.reciprocal_approx_fast(rr, den)
                nc.gpsimd.tensor_mul(
                    oT[:, gq0:gq0 + QC], oT[:, gq0:gq0 + QC], rr)

            def eproj_pieces(tb, at_tail=False):
                """Out-projection chains for token block tb (2 MMs each,
                contracting both groups; bf16 partial to DRAM). At the
                tail the psum->sbuf casts alternate DVE/scalar (the exp
                stream is finished, the scalar engine is idle) and the
                writes spread over all three DMA queues."""
                def chain(tb, ec):
                    ps = pse.tile([128, PCB], F32, tag="pse",
                                  name=f"op{tb}_{ec}")
                    for g in range(NG):
                        nc.tensor.matmul(
                            ps[:, 0:512],
                            lhsT=oT[:, g * seq + tb * 128:
                                    g * seq + (tb + 1) * 128],
                            rhs=wo_s[:, g, ec * 512:(ec + 1) * 512],
                            start=(g == 0), stop=(g == NG - 1),
                        )
                    ob = obp.tile([128, 512], BF16, tag="ob",
                                  name=f"ob{tb}_{ec}")
                    if (tb * 2 + ec) % 2:
                        nc.scalar.copy(out=ob, in_=ps[:, 0:512])
                    else:
                        nc.vector.tensor_copy(out=ob, in_=ps[:, 0:512])
                    if at_tail:
                        eng = (nc.sync, nc.scalar, nc.gpsimd)[(tb * 2 + ec) % 3]
                    else:
                        eng = (nc.sync, nc.gpsimd)[(tb * 2 + ec) % 2]
                    eng.dma_start(
                        out=out_d[tb * 128:(tb + 1) * 128,
                                  ec * 512:(ec + 1) * 512],
                        in_=ob,
                    )
                return [lambda ec=ec: chain(tb, ec) for ec in range(E // 512)]

            # ---------- emission ----------
            for pc in range(NPCB):
                proj_qk(0, pc)
            for st in range(NSTB // 2):
                proj_v(0, st)

            # attention schedule: small chunks first (their PE deficit is
            # covered by g1 projection fillers while the exp stream is
            # short), big exp-dense chunks last (their deep d1 loops give
            # the den-roundtrip of earlier norms plenty of slack before
            # the out-proj chains consume oT). Chains for chunk c are
            # placed >= 2 chunks after (1, c) so they never stall the
            # in-order PE queue.
            spc = QC // 128
            sched = [(0, 1), (1, 1), (0, 0), (1, 0),
                     (0, 3), (1, 3), (0, 2), (1, 2)]
            if NQC != 4:
                sched = []
                for c in reversed(range(NQC)):
                    sched.append((0, c))
                    sched.append((1, c))
            fq_map = {s: [] for s in sched}
            for pc in range(NPCB):
                fq_map[sched[0]].extend(proj_qk_pieces(1, pc))
            for st in range(NSTB):
                fq_map[sched[1]].extend(proj_v_pieces(1, st))
            # v-g0 upper half: only needed from chunk c=2 on; fill the
            # small chunks' thin queues with it
            if NQC == 4:
                for st in range(NSTB // 2, 3 * NSTB // 4):
                    fq_map[(0, 0)].extend(proj_v_pieces(0, st))
                for st in range(3 * NSTB // 4, NSTB):
                    fq_map[(1, 0)].extend(proj_v_pieces(0, st))
            else:
                for st in range(NSTB // 2, NSTB):
                    fq_map[sched[0]].extend(proj_v_pieces(0, st))

            def chains_of(c, at_tail=False):
                out = []
                for tb in range(c * spc, (c + 1) * spc):
                    out.extend(eproj_pieces(tb, at_tail))
                return out

            late_map = {s: [] for s in sched}
            tail = []
            if NQC == 4:
                late_map[(0, 3)].extend(chains_of(1))
                late_map[(1, 3)].extend(chains_of(0))
                late_map[(1, 2)].extend(chains_of(3))
                tail = chains_of(2, at_tail=True)
            else:
                for i, s in enumerate(sched):
                    g, c = s
                    if g != 1:
                        continue
                    chains = chains_of(c)
                    targets = sched[i + 2:i + 4]
                    if not targets:
                        tail.extend(chains)
                    else:
                        per = (len(chains) + len(targets) - 1) // len(targets)
                        for j, ns in enumerate(targets):
                            late_map[ns].extend(chains[j * per:(j + 1) * per])

            # chunk-boundary overlap: the next chunk's first two d1s are
            # emitted BEFORE the previous chunk's final two d2s (which
            # wait on the last exps), so the in-order PE queue never
            # idles across boundaries.
            prev = None
            for s in sched + [None]:
                if s is not None:
                    g, c = s
                    nkt_s = (c * QC + QC) // 128
                    fqs = fq_map[s]
                    for kj in range(2):
                        pts_cache[(g, c, kj)] = d1_kj(g, c, kj)
                        # pop this iteration's filler share too (fillers
                        # can feed this chunk's own d2 stream, e.g. the
                        # g1 V projection)
                        npop = -(-len(fqs) // (nkt_s - kj))
                        for _ in range(npop):
                            if fqs:
                                fqs.pop(0)()
                if prev is not None:
                    pg, pc_, pops, pnkt = prev
                    for kj in range(max(pnkt - 2, 0), pnkt):
                        d2_kj(pg, pc_, kj, pops, pnkt)
                    d3_norm(pg, pc_, pops, last=(s is None))
                if s is None:
                    break
                ops_, nkt = attn_body(g, c, fq_map[s], late_map[s],
                                      start_kj=2)
                prev = (g, c, ops_, nkt)
            for p in tail:
                p()

    nc.compile()
    return nc


@functools.lru_cache(maxsize=2)
def _built(seq: int) -> bacc.Bacc:
    return _build(seq)


def _host_tables(seq: int):
    inv = 1.0 / (ROPE_BASE ** (np.arange(0, HD, 2, dtype=np.float32) / HD))
    f = np.outer(np.arange(seq, dtype=np.float32), inv)
    emb = np.concatenate([f, f], axis=-1)        # [S, 64] (concat layout)
    cos = np.cos(emb).T.astype(np.float32)       # [64, S]
    sin = np.sin(emb).T.astype(np.float32)
    sgn = np.where(np.arange(HD) % 2 == 0, -1.0, 1.0).astype(np.float32)
    sin_signed = sin * sgn[:, None]
    cosT = np.concatenate([cos, cos], axis=0).astype(BF)       # [128, S]
    sinT = np.concatenate([sin_signed, sin_signed], axis=0).astype(BF)
    return cosT, sinT


def make_in_maps(x, Wq, Wk, Wv, Wo):
    x = np.asarray(x, dtype=np.float32)
    B, S, E_ = x.shape
    assert E_ == E
    xTs = [np.ascontiguousarray(x[b].T).astype(BF) for b in range(B)]  # [E,S]
    cosT, sinT = _host_tables(S)
    i_idx = np.arange(128)
    tri = (i_idx[None, :] >= i_idx[:, None]).astype(BF)  # keep j >= i
    scale = np.float32(HD ** -0.5 * EXPK)
    in_maps = []
    for core in range(N_CORES):
        b = core // 4
        m = core % 4
        rows = slice(m * 256, m * 256 + 256)     # this core's 4 heads' dims
        wqT = np.ascontiguousarray((np.asarray(Wq)[rows, :] * scale).T).astype(BF)
        wkT = np.ascontiguousarray(np.asarray(Wk)[rows, :].T).astype(BF)
        wvT = np.ascontiguousarray(np.asarray(Wv)[rows, :].T).astype(BF)
        woT = np.ascontiguousarray(np.asarray(Wo)[:, rows].T).astype(BF)
        in_maps.append(dict(
            xT=xTs[b], wqT=wqT, wkT=wkT, wvT=wvT, woT=woT,
            cosT=cosT, sinT=sinT, tri=tri,
        ))
    return in_maps


def kernel(x, Wq, Wk, Wv, Wo):
    x = np.asarray(x, dtype=np.float32)
    B, S, E_ = x.shape
    nc = _built(S)
    in_maps = make_in_maps(x, Wq, Wk, Wv, Wo)
    res = run_bass_kernel_spmd(nc, in_maps, core_ids=list(range(N_CORES)))
    out = np.zeros((B, S, E_), np.float32)
    for core, r in enumerate(res.results):
        out[core // 4] += r["out_p"].astype(np.float32)
    return out
